# revision 1
# baseline (speedup 1.0000x reference)
"""Trainium2 Bass kernel for ErosionP4 (P4 group-equivariant grayscale erosion).

Reference computation (shapes hardcoded):
  x: [B=4, G=4, H=96, W=96, C=4] fp32, kernel: [5, 5, 3, C=4, F=8] fp32
  out[b,g,h,w,f] = sum_c min_{k,dy,dx} ( ygp[b,g,k,h+dy,w+dx,c] - krev[g,dy,dx,k,c,f] )
  where ygp[b,g,k] = x[b, (g+k-1) mod 4] spatially padded with +inf and
  krev = the 4 planar rotations of the depth-rotated SE, spatially reversed.

Sharding: core -> (g = core//2, f-half = core%2).  Each core computes all 4
batches for one group-rotation g and 4 of the 8 filters.  All four batches
share the SE values for the core's g.

Packing "cp128": the (c, h) axes are flattened into a 384-row stream split
into 3 chunks of 128 partitions, so every DVE instruction runs with all 128
lanes busy.  The per-(tap,c,f) SE value varies across partitions within a
chunk, carried by the per-partition scalar operand.  The channel sum then
happens on the host (the c pieces are partition-misaligned on device).

Per (tap, f, chunk) the erosion update acc = min(window - kk, acc) runs as
two DVE ops — tensor_scalar subtract (4x fp16 uop) + tensor_tensor min (2x
fp16 uop), HW-measured at 373 us vs 470 us for the fused 1x
scalar_tensor_tensor (CFG_SPLIT=0 fallback).
"""

import os
from contextlib import ExitStack

import numpy as np

import concourse.bass as bass
import concourse.mybir as mybir
import concourse.tile as tile
from concourse.bass_utils import run_bass_kernel_spmd

B, G, H, W, C = 4, 4, 96, 96, 4
KH, KW, F = 5, 5, 8
PAD = 2
HP, WP = H + PAD * 2, W + PAD * 2  # 100, 100
NTAP = 3 * KH * KW  # 75
N_CORES = 8
NP = 4  # batches per core
NF = F // 2  # filters per core
NCHUNK = 3  # ceil(C*H / 128)

# Configuration (module-level so experiments can flip them; defaults = best).
CFG_DTYPE = os.environ.get("KCFG_DTYPE", "fp16")  # fp32 | fp16 | bf16
CFG_PACK = os.environ.get("KCFG_PACK", "cp128")  # h96 | cp128
CFG_GPSIMD = int(os.environ.get("KCFG_GPSIMD", "0"))  # of NF*NCHUNK (cp128) or C*NF (h96) columns on gpsimd
CFG_REPEAT = int(os.environ.get("KCFG_REPEAT", "1"))  # repeat compute on-device (timing slope runs)
CFG_SPLIT = int(os.environ.get("KCFG_SPLIT", "1"))  # 1: unfused ts+tt (2x/4x uops); 0: fused scalar_tensor_tensor
CFG_ACTSUB = int(os.environ.get("KCFG_ACTSUB", "0"))  # cols whose subtract runs on the Scalar engine

_DT = {
    "fp32": (mybir.dt.float32, np.float32, 1e30),
    "fp16": (mybir.dt.float16, np.float16, 30000.0),
    "bf16": (mybir.dt.bfloat16, None, 1e30),
}

_prog_cache = {}
LAST_RESULTS = None


def _np_dtype(name):
    if name == "bf16":
        import ml_dtypes

        return np.dtype(ml_dtypes.bfloat16)
    return np.dtype(_DT[name][1])


def _chunk_ranges(m):
    """(c, h0, h1, p0, p1) pieces of stream rows [128m, 128(m+1))."""
    out = []
    r = 128 * m
    while r < 128 * (m + 1):
        c, h = r // H, r % H
        h1 = min(H, h + 128 * (m + 1) - r)
        out.append((c, h, h1, r - 128 * m, r - 128 * m + (h1 - h)))
        r += h1 - h
    return out


def _build_program(dtype_name, pack, gpsimd_n, repeat=1):
    dt, _, _ = _DT[dtype_name]
    two_byte = dtype_name in ("fp16", "bf16")
    # The kernel-tail Drain must wait on every sem lane used; with 8 SWDGE
    # lanes + 3 engines it exceeds the CTRL struct's sync-wait capacity.
    # Cap the SWDGE completion-sem lanes for this build.
    import concourse.tile_sem_assignment as _tsa

    _orig_swdge = _tsa.NUM_SWDGE_GLOBAL_SEMS
    _tsa.NUM_SWDGE_GLOBAL_SEMS = 4
    try:
        return _build_program_inner(dtype_name, pack, gpsimd_n, dt, two_byte, repeat)
    finally:
        _tsa.NUM_SWDGE_GLOBAL_SEMS = _orig_swdge


class _SplitDrainTC(tile.TileContext):
    """TileContext whose kernel-tail drain is split into one drain per sem
    lane: the stock single Drain carries a wait for every lane used, which
    overflows the CTRL struct's sync-wait encoding on this compiler."""

    def _drain_and_barrier(self, tick_clock, wait_clock):
        from concourse.tile_sem_assignment import N_PROCS
        from concourse.vector_clock import ScopedClock, VectorClock

        gc = tick_clock.global_clock
        ticks = [gc[p] for p in range(N_PROCS)]
        for p in range(N_PROCS):
            if ticks[p] <= 0:
                continue
            sub = [ticks[q] if q == p else 0 for q in range(N_PROCS)]
            d = self.nc.sync.drain()
            wait_clock.add_sem_waits(d.ins, ScopedClock({None: VectorClock(sub)}))

        self.nc.all_engine_barrier()
        assert self.sems is not None
        popped = self.nc._tile_sem_poison_stack.pop()
        assert popped is self._sem_poison
        self.nc.clear_and_free_semaphores(list(self.sems.allocated().values()))
        self.nc.all_engine_barrier()


def _build_program_inner(dtype_name, pack, gpsimd_n, dt, two_byte, repeat=1):
    nc = bass.Bass()
    # Input planes: [k, c, h_pad, pair, w_pad]; for 2-byte dtypes a second
    # copy shifted by one w element keeps odd-dx windows 4B-aligned (DVE
    # 2x packed mode needs aligned step-1 operands).
    xin = nc.declare_dram_parameter("xin", [3, C, HP, NP, WP], dt, isOutput=False)

    if pack == "cp128":
        ncols = NF * NCHUNK  # engine-split granularity per tap
        nkk = NTAP * NF * NCHUNK
        kkin = nc.declare_dram_parameter("kk", [128, 2 * nkk], mybir.dt.float32, isOutput=False)
        yout = nc.declare_dram_parameter("yout", [NF, 128, NCHUNK, NP, W], dt, isOutput=True)
    else:
        ncols = C * NF
        nkk = NTAP * ncols
        kkin = nc.declare_dram_parameter("kk", [H, nkk], mybir.dt.float32, isOutput=False)
        yout = nc.declare_dram_parameter("yout", [H, NP, W, NF], mybir.dt.float32, isOutput=True)

    with _SplitDrainTC(nc) as tc, ExitStack() as ctx:
        pool = ctx.enter_context(tc.tile_pool(name="main", bufs=1))

        # Compute-instruction ISA slots can encode only ONE sync wait, so
        # "touch" every DMA'd region with a trivial op on each consuming
        # engine right after its DMA (one wait each); later compute
        # instructions then inherit the dependency through engine program
        # order and carry no waits of their own.
        # Distinct destination slots per touch: a shared destination would be a
        # same-engine WAW hazard, which costs this instruction's single wait slot.
        touch_v = pool.tile([1, 512], mybir.dt.float32, name="touch_v", tag="touch_v")
        touch_s = pool.tile([1, 512], mybir.dt.float32, name="touch_s", tag="touch_s")
        touch_g = pool.tile([1, 512], mybir.dt.float32, name="touch_g", tag="touch_g")
        tctr = [0, 0, 0]

        def _touch(t, p0=0, scalar_too=False):
            src = t[p0 : p0 + 1, 0:1]
            i = tctr[0] = tctr[0] + 1
            nc.vector.tensor_scalar_add(touch_v[0:1, i : i + 1], src, 0.0)
            if scalar_too and two_byte:
                i = tctr[1] = tctr[1] + 1
                nc.scalar.copy(touch_s[0:1, i : i + 1], src)
            i = tctr[2] = tctr[2] + 1
            nc.gpsimd.tensor_scalar_add(touch_g[0:1, i : i + 1], src, 0.0)

        NPART = 128 if pack == "cp128" else H

        # One HWDGE dma_start fans out over several HW queues, so a consumer
        # would need more sync waits than compute-instruction ISA slots can
        # encode; the software DGE (gpsimd engine) uses a single queue.
        dma = nc.gpsimd.dma_start

        # Compute-engine SBUF reads must start at partition 0/32/64/96, so the
        # dy window shift cannot be a partition offset — keep one dy-shifted
        # copy per (k, dy, chunk) (cp128) / (k, dy, c) (h96), sliced from HBM.
        # The odd-dx alignment copy (in_b = in_a shifted one element left) is
        # built by the otherwise-idle Scalar engine instead of more DMAs.
        in_a = {}
        in_b = {}

        def _load(key, src_k, src_c_or_ranges, dy):
            if pack == "cp128":
                t = pool.tile([128, NP, WP], dt, name=f"ina_{key}", tag=f"ina_{key}")
                for (c, h0, h1, p0, p1) in src_c_or_ranges:
                    dma(t[p0:p1], xin[src_k, c, h0 + dy : h1 + dy])
                    _touch(t[:, 0], p0, scalar_too=True)
            else:
                t = pool.tile([H, NP, WP], dt, name=f"ina_{key}", tag=f"ina_{key}")
                dma(t[:], xin[src_k, src_c_or_ranges, dy : dy + H])
                _touch(t[:, 0], 0, scalar_too=True)
            in_a[key] = t
            if two_byte:
                tb = pool.tile(list(t.shape), dt, name=f"inb_{key}", tag=f"inb_{key}")
                nc.scalar.copy(tb[:, :, 0 : WP - 1], t[:, :, 1:WP])
                _touch(tb[:, 0])
                in_b[key] = tb

        for k in range(3):
            for dy in range(KH):
                if pack == "cp128":
                    for m in range(NCHUNK):
                        _load((k, dy, m), k, _chunk_ranges(m), dy)
                else:
                    for c in range(C):
                        _load((k, dy, c), k, c, dy)

        kkt = pool.tile([NPART, 2 * nkk if pack == "cp128" else nkk], mybir.dt.float32, name="kkt", tag="kkt")
        dma(kkt[:], kkin[:])
        _touch(kkt, 0, scalar_too=True)

        accs = {}
        if pack == "cp128":
            # One tile per filter with the chunk index as a free dim, so the
            # output needs only NF=4 DMAs (<=8 HWDGE queues, no FIFO reuse wait).
            accf = {}
            for f in range(NF):
                accf[f] = pool.tile([128, NCHUNK, NP, W], dt, name=f"acc_{f}", tag=f"acc_{f}")
                for m in range(NCHUNK):
                    accs[f, m] = accf[f][:, m]
        else:
            for c in range(C):
                for f in range(NF):
                    accs[c, f] = pool.tile([H, NP, W], dt, name=f"acc_{c}_{f}", tag=f"acc_{c}_{f}")

        taps = [(k, dy, dx) for k in range(3) for dy in range(KH) for dx in range(KW)]

        # Unfused two-op path: tensor_scalar has a 4x fp16 uop and plain
        # tensor_tensor min a 2x one, while the fused scalar_tensor_tensor
        # only runs 1x — two instructions are cheaper than one.  gpsimd (no
        # scalar_tensor_tensor support) uses the same two-op shape.
        split_ops = two_byte and pack == "cp128" and CFG_SPLIT
        n_act = CFG_ACTSUB if split_ops else 0
        # Full-width tmp tiles: the three chunk subtracts land in one tile so
        # a single 2x tensor_tensor min (FD=1152) covers all chunks of a
        # filter, amortizing the per-instruction SBUF bubble.
        tmp_w = [pool.tile([128, NCHUNK, NP, W], dt, name=f"tmpw_{i}", tag=f"tmpw_{i}") for i in range(4)] if split_ops else []
        tmp_v = [pool.tile([128, NP, W], dt, name=f"tmpv_{i}", tag=f"tmpv_{i}") for i in range(4)] if split_ops else []
        tmp_g = [pool.tile([128, NP, W], dt, name=f"tmpg_{i}", tag=f"tmpg_{i}") for i in range(4)] if (split_ops and gpsimd_n > 0) else []
        tmp_a = [pool.tile([128, NP, W], dt, name=f"tmpa_{i}", tag=f"tmpa_{i}") for i in range(16)] if n_act else []
        actr = [0]

        def emit(ti, win_sel, acc, col):
            kk_ap = kkt[:, ti * ncols + col : ti * ncols + col + 1]
            on_gp = col >= ncols - gpsimd_n
            on_act = (not on_gp) and n_act > 0 and col >= ncols - gpsimd_n - n_act
            eng = nc.gpsimd if on_gp else nc.vector
            if ti == 0:
                eng.tensor_scalar(acc[:], win_sel, kk_ap, None, mybir.AluOpType.subtract)
            elif split_ops:
                if on_act:
                    # ACT computes win - kk via its per-partition bias (the
                    # negated kk in the second kk half); DVE keeps only the
                    # 2x tensor_tensor min.
                    negkk_ap = kkt[:, nkk + ti * ncols + col : nkk + ti * ncols + col + 1]
                    tmp = tmp_a[actr[0] % 16]
                    actr[0] += 1
                    nc.scalar.activation(
                        tmp[:], win_sel, mybir.ActivationFunctionType.Identity, bias=negkk_ap
                    )
                    nc.vector.tensor_tensor(acc[:], tmp[:], acc[:], mybir.AluOpType.min)
                else:
                    tmp = (tmp_g if on_gp else tmp_v)[col % 4]
                    eng.tensor_scalar(tmp[:], win_sel, kk_ap, None, mybir.AluOpType.subtract)
                    eng.tensor_tensor(acc[:], tmp[:], acc[:], mybir.AluOpType.min)
            else:
                eng.scalar_tensor_tensor(
                    acc[:], win_sel, kk_ap, acc[:],
                    mybir.AluOpType.subtract, mybir.AluOpType.min,
                )

        for _rep in range(repeat):
          for ti, (k, dy, dx) in enumerate(taps):
            use_b = two_byte and (dx % 2 == 1)
            dxa = dx - 1 if use_b else dx
            if pack == "cp128":
                if n_act and ti > 0:
                    # ACT absorber: observe DVE's latest acc tick so ACT's ring
                    # rewrites carry only their same-engine WAW wait.
                    i = tctr[1] = tctr[1] + 1
                    nc.scalar.copy(touch_s[0:1, i : i + 1], accs[NF - 1, NCHUNK - 1][0:1, 0, 0:1])
                    # emit this tap's ACT subs first, then a DVE absorber on the
                    # last one so the tt-mins carry only their acc-chain wait.
                    for f in range(NF):
                        for m in range(NCHUNK):
                            col = f * NCHUNK + m
                            if col >= ncols - gpsimd_n - n_act and col < ncols - gpsimd_n:
                                negkk_ap = kkt[:, nkk + ti * ncols + col : nkk + ti * ncols + col + 1]
                                tmp = tmp_a[actr[0] % 16]
                                actr[0] += 1
                                srct = in_b[k, dy, m] if use_b else in_a[k, dy, m]
                                nc.scalar.activation(
                                    tmp[:], srct[:, :, dxa : dxa + W],
                                    mybir.ActivationFunctionType.Identity, bias=negkk_ap,
                                )
                    i = tctr[0] = tctr[0] + 1
                    nc.vector.tensor_scalar_add(
                        touch_v[0:1, i : i + 1], tmp_a[(actr[0] - 1) % 16][0:1, 0, 0:1], 0.0
                    )
                    for f in range(NF):
                        for m in range(NCHUNK):
                            col = f * NCHUNK + m
                            if col >= ncols - gpsimd_n - n_act and col < ncols - gpsimd_n:
                                tmp = tmp_a[(actr[0] - (ncols - gpsimd_n - (ncols - gpsimd_n - n_act)) + (col - (ncols - gpsimd_n - n_act))) % 16]
                                nc.vector.tensor_tensor(accs[f, m][:], tmp[:], accs[f, m][:], mybir.AluOpType.min)
                            elif col < ncols - gpsimd_n - n_act or col >= ncols - gpsimd_n:
                                srct = in_b[k, dy, m] if use_b else in_a[k, dy, m]
                                emit(ti, srct[:, :, dxa : dxa + W], accs[f, m], col)
                elif split_ops and gpsimd_n == 0 and ti > 0:
                    # merged form: per filter, 3 chunk subtracts into one tmp
                    # tile, then one full-width tensor_tensor min.
                    for f in range(NF):
                        tmp = tmp_w[(ti * NF + f) % 4]
                        for m in range(NCHUNK):
                            col = f * NCHUNK + m
                            kk_ap = kkt[:, ti * ncols + col : ti * ncols + col + 1]
                            srct = in_b[k, dy, m] if use_b else in_a[k, dy, m]
                            nc.vector.tensor_scalar(
                                tmp[:, m], srct[:, :, dxa : dxa + W], kk_ap, None,
                                mybir.AluOpType.subtract,
                            )
                        nc.vector.tensor_tensor(
                            accf[f][:], tmp[:], accf[f][:], mybir.AluOpType.min
                        )
                else:
                    for f in range(NF):
                        for m in range(NCHUNK):
                            src = in_b[k, dy, m] if use_b else in_a[k, dy, m]
                            emit(ti, src[:, :, dxa : dxa + W], accs[f, m], f * NCHUNK + m)
            else:
                for c in range(C):
                    src = in_b[k, dy, c] if use_b else in_a[k, dy, c]
                    win = src[:, :, dxa : dxa + W]
                    for f in range(NF):
                        emit(ti, win, accs[c, f], c * NF + f)

        if pack == "cp128":
            # Channel sum happens on the host; just store the 12 acc tiles.
            for f in range(NF):
                # A Pool-engine touch absorbs the DVE dependency (1 wait), so
                # the SWDGE out-DMA dispatched next on the same sequencer needs
                # only its queue-FIFO wait.
                i = tctr[2] = tctr[2] + 1
                nc.gpsimd.tensor_scalar_add(touch_g[0:1, i : i + 1], accf[f][0:1, 0, 0, 0:1], 0.0)
                dma(yout[f], accf[f][:])

        else:
            out_t = pool.tile([H, NP, W, NF], mybir.dt.float32, name="out_t", tag="out_t")
            for f in range(NF):
                s1 = pool.tile([H, NP, W], mybir.dt.float32, name=f"s1_{f}", tag="s1", bufs=2)
                s2 = pool.tile([H, NP, W], mybir.dt.float32, name=f"s2_{f}", tag="s2", bufs=2)
                nc.vector.tensor_add(s1[:], accs[0, f][:], accs[1, f][:])
                nc.vector.tensor_add(s2[:], accs[2, f][:], accs[3, f][:])
                nc.vector.tensor_add(out_t[:, :, :, f], s1[:], s2[:])
            nc.sync.dma_start(yout[:], out_t[:])

    return nc


def _get_program(dtype_name, pack, gpsimd_n, repeat=1):
    key = (dtype_name, pack, gpsimd_n, repeat, CFG_SPLIT, CFG_ACTSUB)
    if key not in _prog_cache:
        _prog_cache[key] = _build_program(dtype_name, pack, gpsimd_n, repeat)
    return _prog_cache[key]


def _krev(kernel):
    """[g, dy, dx, k, c, f] rotated/reversed SE, pure re-indexing of `kernel`."""
    k_ero = np.stack(
        [
            np.rot90(kernel[:, :, 2], k=3, axes=(0, 1)),
            kernel[:, :, 1],
            np.rot90(kernel[:, :, 0], k=1, axes=(0, 1)),
        ],
        axis=2,
    )
    krot = np.stack([np.rot90(k_ero, k=j, axes=(0, 1)) for j in range(4)], axis=0)
    return krot[:, ::-1, ::-1]


def _core_units(core):
    g = core // 2
    fh = core % 2
    return g, list(range(B)), list(range(fh * NF, fh * NF + NF))


def _make_in_map(x, kr, pack, core, np_dt, big, two_byte):
    g, bs, fs = _core_units(core)
    planes = np.full((3, C, HP, NP, WP), big, np.float32)
    for pi, b in enumerate(bs):
        for k in range(3):
            src = x[b, (g + k - 1) % 4]  # [H, W, C]
            planes[k, :, PAD : PAD + H, pi, PAD : PAD + W] = src.transpose(2, 0, 1)
    sel = kr[g][:, :, :, :, fs]  # [dy, dx, k, c, NF]
    taps_kcf = np.ascontiguousarray(sel.transpose(2, 0, 1, 3, 4))  # [k,dy,dx,c,NF]
    if pack == "cp128":
        # kk[p, (tap, f, m)] = kr[g, tap, c(m,p), f]
        tap_cf = taps_kcf.reshape(NTAP, C, NF)
        kk = np.empty((128, NTAP * NF * NCHUNK), np.float32)
        for m in range(NCHUNK):
            for (c, h0, h1, p0, p1) in _chunk_ranges(m):
                for ti in range(NTAP):
                    for f in range(NF):
                        kk[p0:p1, (ti * NF + f) * NCHUNK + m] = tap_cf[ti, c, f]
        kk = np.concatenate([kk, -kk], axis=1)
    else:
        kkflat = taps_kcf.reshape(-1)
        kk = np.ascontiguousarray(np.broadcast_to(kkflat, (H, kkflat.size)))
    return {"xin": planes.astype(np_dt), "kk": np.ascontiguousarray(kk)}


def _assemble(results, pack):
    out = np.zeros((B, G, H, W, F), np.float32)
    for core in range(N_CORES):
        g, bs, fs = _core_units(core)
        y = np.asarray(results[core]["yout"]).astype(np.float32)
        if pack == "cp128":
            # y: [NF, NCHUNK, 128, NP, W]; sum the c pieces into out
            for fi, f in enumerate(fs):
                for m in range(NCHUNK):
                    for (c, h0, h1, p0, p1) in _chunk_ranges(m):
                        for pi, b in enumerate(bs):
                            out[b, g, h0:h1, :, f] += y[fi, p0:p1, m, pi, :]
        else:
            for pi, b in enumerate(bs):
                out[b, g, :, :, fs[0] : fs[0] + len(fs)] = y[:, pi]
    return out


def kernel(x, kernel):
    x = np.ascontiguousarray(np.asarray(x, dtype=np.float32))
    se = np.ascontiguousarray(np.asarray(kernel, dtype=np.float32))
    dtype_name, pack, gpsimd_n = CFG_DTYPE, CFG_PACK, CFG_GPSIMD
    np_dt = _np_dtype(dtype_name)
    big = _DT[dtype_name][2]
    two_byte = dtype_name in ("fp16", "bf16")

    kr = _krev(se)  # [g, dy, dx, k, c, f]
    in_maps = [
        _make_in_map(x, kr, pack, core, np_dt, big, two_byte) for core in range(N_CORES)
    ]

    nc = _get_program(dtype_name, pack, gpsimd_n, CFG_REPEAT)
    res = run_bass_kernel_spmd(nc, in_maps, list(range(N_CORES)), trace=False)
    global LAST_RESULTS
    LAST_RESULTS = res
    return _assemble(res.results, pack)



# revision 14
# speedup vs baseline: 3.0105x; 3.0105x over previous
"""Trainium2 Bass kernel for ErosionP4 (P4 group-equivariant grayscale erosion).

Reference computation (shapes hardcoded):
  x: [B=4, G=4, H=96, W=96, C=4] fp32, kernel: [5, 5, 3, C=4, F=8] fp32
  out[b,g,h,w,f] = sum_c min_{k,dy,dx} ( ygp[b,g,k,h+dy,w+dx,c] - krev[g,dy,dx,k,c,f] )
  where ygp[b,g,k] = x[b, (g+k-1) mod 4] spatially padded with +inf and
  krev = the 4 planar rotations of the depth-rotated SE, spatially reversed.

Sharding: core -> (g = core//2, f-half = core%2).  Each core computes all 4
batches for one group-rotation g and 4 of the 8 filters.

Layout "c-block": partition p = 32*c + hs (c = channel, hs = h mod 32), free
dims (hb, b, w) with h = 32*hb + hs.  Each partition sees exactly one channel,
so the per-(tap, f) SE value is a per-partition scalar for FULL-width
tensor_scalar instructions (no chunk splitting).  The channel sum happens on
the host (c pieces live on different partitions).

The 15 (k, dy)-shifted input planes are pre-built on the host and DMA'd as 15
contiguous tiles via HWDGE on the otherwise-idle SP engine (SWDGE descriptor
generation would burn the Pool engine, which now does compute).

The 75 taps x 4 filters of acc = min(acc, window - kk) are split across three
engines (greedy balance by simulated per-engine cost):
  - DVE-own taps (even dx only, for 4B-aligned 4x packed reads):
      4x tensor_scalar subtract (4x mode) into a private tmp + one full-width
      (4 filters at once) tensor_tensor min (2x mode) into acc_dve.
  - ACT-assist taps: 4x activation(Identity, bias=-kk) subs into a ring slot,
      DVE does the full-width min into acc_dve (one cross-engine wait).
  - Pool taps: 4x fused scalar_tensor_tensor (subtract, min) into acc_pool
      (the Q7 software implementation fuses both ALU stages at no extra cost).
Finally acc = min(acc_dve, acc_pool) on DVE and one HWDGE store of the
[128, 4, 1152] fp16 result.
"""

import os
from contextlib import ExitStack

import numpy as np

import concourse.bass as bass
import concourse.mybir as mybir
import concourse.tile as tile
from concourse.bass_utils import run_bass_kernel_spmd

B, G, H, W, C = 4, 4, 96, 96, 4
KH, KW, F = 5, 5, 8
PAD = 2
WP = W + PAD * 2  # 100
NTAP = 3 * KH * KW  # 75
N_CORES = 8
NP = 4  # batches per core
NF = F // 2  # filters per core
HS = 32  # h rows per (c, hb) block
HB = 3  # h blocks
BIG = 30000.0  # +inf stand-in that survives fp16

CFG_REPEAT = int(os.environ.get("KCFG_REPEAT", "1"))
# simulated per-unit costs (ns) used by the static scheduler.  The Pool
# engine is DMA-only on this walrus backend (all its elementwise tensor ops
# fail the codegen engine check), so compute is split DVE vs ACT only.
COST_DVE_SUB = 360.0
COST_DVE_MIN_WIDE = 2460.0
COST_ACT_SUB = 1185.0  # activation + amortized absorber copies

FP16 = mybir.dt.float16

_prog_cache = {}
LAST_RESULTS = None


def _taps():
    return [(k, dy, dx) for k in range(3) for dy in range(KH) for dx in range(KW)]


def _schedule():
    """Greedy static assignment of the 75 taps to engines.

    Returns list of 'dve' | 'act' | 'pool' per tap index.  DVE-own taps are
    restricted to even dx (aligned 4x packed reads); the first tap must be
    'dve' so acc_dve exists before any assist-min, and the first 'pool' tap
    initializes acc_pool.
    """
    taps = _taps()
    t_dve = t_act = 0.0
    out = []
    for ti, (k, dy, dx) in enumerate(taps):
        # projected end-times if this tap went to each engine
        cand = []
        if dx % 2 == 0:
            cand.append(("dve", t_dve + NF * COST_DVE_SUB + COST_DVE_MIN_WIDE))
        cand.append(("act", max(t_act + NF * COST_ACT_SUB,
                                t_dve + COST_DVE_MIN_WIDE)))
        if ti == 0:
            cand = [c for c in cand if c[0] == "dve"]
        eng, _ = min(cand, key=lambda c: c[1])
        if eng == "dve":
            t_dve += NF * COST_DVE_SUB + COST_DVE_MIN_WIDE
        else:
            t_act = max(t_act, 0.0) + NF * COST_ACT_SUB
            t_dve += COST_DVE_MIN_WIDE
        out.append(eng)
    return out


class _SplitDrainTC(tile.TileContext):
    """TileContext whose kernel-tail drain is split into one drain per sem
    lane: the stock single Drain carries a wait for every lane used, which
    overflows the CTRL struct's sync-wait encoding on this compiler."""

    def _drain_and_barrier(self, tick_clock, wait_clock):
        from concourse.tile_sem_assignment import N_PROCS
        from concourse.vector_clock import ScopedClock, VectorClock

        gc = tick_clock.global_clock
        ticks = [gc[p] for p in range(N_PROCS)]
        for p in range(N_PROCS):
            if ticks[p] <= 0:
                continue
            sub = [ticks[q] if q == p else 0 for q in range(N_PROCS)]
            d = self.nc.sync.drain()
            wait_clock.add_sem_waits(d.ins, ScopedClock({None: VectorClock(sub)}))

        self.nc.all_engine_barrier()
        assert self.sems is not None
        popped = self.nc._tile_sem_poison_stack.pop()
        assert popped is self._sem_poison
        self.nc.clear_and_free_semaphores(list(self.sems.allocated().values()))
        self.nc.all_engine_barrier()


def _strip_stale_same_engine_waits(nc, lag=8):
    """Drop same-engine sem waits whose producer finished >= `lag` own-engine
    instructions earlier.

    This tile version emits a sem wait for EVERY hazard, including same-engine
    WAW/WAR whose producers are long retired; compute ISA structs can encode
    only ONE sync wait, so a ring-buffer rewrite (same-engine WAW + cross-
    engine WAR) overflows codegen.  Engines issue in order and their writes
    land within a couple of instructions, so a same-engine wait on a producer
    `lag` instructions back is vacuous.  Recent same-engine waits (pipelined
    RAW guards) are kept.
    """
    strip_types = {
        "InstActivation", "InstTensorScalarPtr", "InstTensorTensor",
        "InstTensorScalar", "InstMemset", "InstCopy", "InstTensorCopy",
        "InstTensorReduce",
    }
    counts = {}
    fn = nc.m.functions[0]
    for blk in fn.blocks:
        for ins in blk.instructions:
            si = ins.sync_info
            if si is None:
                continue
            eng = getattr(ins, "engine", None)
            ename = getattr(eng, "name", None) or (str(eng).split(".")[-1] if eng else "")
            if si.on_wait and type(ins).__name__ in strip_types and ename in (
                "Activation", "DVE", "Pool", "PE"
            ):
                keep = []
                for w in si.on_wait:
                    nm = w.ant_name or ""
                    if (
                        nm.startswith(ename + "_")
                        and w.wait_mode == "sem-ge-imm"
                        and w.wait_value is not None
                        and counts.get(nm, 0) - w.wait_value >= lag
                    ):
                        continue
                    keep.append(w)
                if len(keep) != len(si.on_wait):
                    si.on_wait = keep
            for u in si.on_update or []:
                if u.ant_name:
                    counts[u.ant_name] = counts.get(u.ant_name, 0) + (u.update_value or 1)
    return nc


def _build_program(repeat=1):
    import concourse.tile_sem_assignment as _tsa

    _orig_swdge = _tsa.NUM_SWDGE_GLOBAL_SEMS
    _tsa.NUM_SWDGE_GLOBAL_SEMS = 4
    try:
        return _strip_stale_same_engine_waits(_build_program_inner(repeat))
    finally:
        _tsa.NUM_SWDGE_GLOBAL_SEMS = _orig_swdge


def _build_program_inner(repeat=1):
    nc = bass.Bass()
    sched = _schedule()
    taps = _taps()

    # xin2[t15][p][hb][b][wp]: host-pre-shifted planes, one contiguous tile per
    # (k, dy).  kk: +kk columns then -kk columns (ACT bias), col = ti*NF + fi.
    xin = nc.declare_dram_parameter("xin", [15, 128, HB, NP, WP], FP16, isOutput=False)
    kkin = nc.declare_dram_parameter("kk", [128, 2 * NTAP * NF], mybir.dt.float32, isOutput=False)
    yout = nc.declare_dram_parameter("yout", [128, NF, HB, NP, W], FP16, isOutput=True)

    with _SplitDrainTC(nc) as tc, ExitStack() as ctx:
        pool = ctx.enter_context(tc.tile_pool(name="main", bufs=1))

        # Compute-instruction ISA slots can encode only ONE sync wait, so
        # "touch" every DMA'd region with a trivial op on each consuming
        # engine right after its DMA; later compute instructions inherit the
        # dependency through engine program order and carry no waits.
        touch_v = pool.tile([1, 2048], mybir.dt.float32, name="touch_v", tag="touch_v")
        touch_s = pool.tile([1, 2048], mybir.dt.float32, name="touch_s", tag="touch_s")
        touch_g = pool.tile([1, 2048], mybir.dt.float32, name="touch_g", tag="touch_g")
        tctr = [0, 0, 0]

        def _touch(src, engines):
            if "v" in engines:
                i = tctr[0] = tctr[0] + 1
                nc.vector.tensor_scalar_add(touch_v[0:1, i : i + 1], src, 0.0)
            if "s" in engines:
                i = tctr[1] = tctr[1] + 1
                nc.scalar.copy(touch_s[0:1, i : i + 1], src)
            if "g" in engines:
                i = tctr[2] = tctr[2] + 1
                nc.gpsimd.tensor_scalar_add(touch_g[0:1, i : i + 1], src, 0.0)

        # which engines read each (k, dy) tile
        tile_readers = {}
        for ti, (k, dy, dx) in enumerate(taps):
            e = {"dve": "v", "act": "s", "pool": "g"}[sched[ti]]
            tile_readers.setdefault((k, dy), set()).add(e)

        in_t = {}
        for k in range(3):
            for dy in range(KH):
                t = pool.tile([128, HB, NP, WP], FP16, name=f"in_{k}_{dy}", tag=f"in_{k}_{dy}")
                nc.sync.dma_start(t[:], xin[(k * KH + dy)])
                _touch(t[0:1, 0, 0, 0:1], tile_readers[(k, dy)])
                in_t[k, dy] = t

        kkt = pool.tile([128, 2 * NTAP * NF], mybir.dt.float32, name="kkt", tag="kkt")
        nc.sync.dma_start(kkt[:], kkin[:])
        _touch(kkt[0:1, 0:1], {"v", "s", "g"})

        acc_d = pool.tile([128, NF, HB, NP, W], FP16, name="acc_d", tag="acc_d")
        acc_p = pool.tile([128, NF, HB, NP, W], FP16, name="acc_p", tag="acc_p")
        tmp_d = pool.tile([128, NF, HB, NP, W], FP16, name="tmp_d", tag="tmp_d")
        NRING = 6
        ring = [
            pool.tile([128, NF, HB, NP, W], FP16, name=f"ring_{i}", tag=f"ring_{i}")
            for i in range(NRING)
        ]

        have_pool_acc = any(e == "pool" for e in sched)

        ring_i = 0
        assist_marks = []
        for _rep in range(repeat):
            first_pool = True
            # touch_v column written by the DVE mark after each assist-min;
            # the ACT absorber for a recycled ring slot reads the mark of the
            # min that read that slot (cross-engine RAW, one wait on a copy),
            # avoiding any ACT read of acc_d (which would put a WAR wait on
            # the next acc-writing TT, whose struct has no room for it).
            for ti, (k, dy, dx) in enumerate(taps):
                eng = sched[ti]
                src = in_t[k, dy]
                win = src[:, :, :, dx : dx + W]
                if eng == "dve":
                    dst = acc_d if ti == 0 else tmp_d
                    for fi in range(NF):
                        kk_ap = kkt[:, ti * NF + fi : ti * NF + fi + 1]
                        nc.vector.tensor_scalar(
                            dst[:, fi], win, kk_ap, None, mybir.AluOpType.subtract
                        )
                    if ti != 0:
                        nc.vector.tensor_tensor(
                            acc_d[:], tmp_d[:], acc_d[:], mybir.AluOpType.min
                        )
                elif eng == "act":
                    slot = ring[ring_i % NRING]
                    if ring_i >= NRING:
                        # ACT absorber: observe the DVE mark of the min that
                        # read this slot, so the ring rewrites carry no WAR
                        # waits (the AC struct cannot encode them).
                        mark_col = assist_marks[ring_i - NRING]
                        _touch(touch_v[0:1, mark_col : mark_col + 1], {"s"})
                    ring_i += 1
                    for fi in range(NF):
                        negkk_ap = kkt[:, NTAP * NF + ti * NF + fi : NTAP * NF + ti * NF + fi + 1]
                        nc.scalar.activation(
                            slot[:, fi], win,
                            mybir.ActivationFunctionType.Identity, bias=negkk_ap,
                        )
                    # DVE absorber on the last sub: the wide min then inherits
                    # all four slice deps through DVE program order (TT structs
                    # encode only one sync wait).
                    _touch(slot[0:1, NF - 1, 0, 0, 0:1], {"v"})
                    nc.vector.tensor_tensor(
                        acc_d[:], slot[:], acc_d[:], mybir.AluOpType.min
                    )
                    # DVE mark: same-engine read of acc_d right after the min
                    i = tctr[0] = tctr[0] + 1
                    nc.vector.tensor_scalar_add(
                        touch_v[0:1, i : i + 1], acc_d[0:1, 0, 0, 0, 0:1], 0.0
                    )
                    assist_marks.append(i)
                else:  # pool
                    for fi in range(NF):
                        kk_ap = kkt[:, ti * NF + fi : ti * NF + fi + 1]
                        if first_pool:
                            nc.gpsimd.tensor_scalar(
                                acc_p[:, fi], win, kk_ap, None, mybir.AluOpType.subtract
                            )
                        else:
                            nc.gpsimd.scalar_tensor_tensor(
                                acc_p[:, fi], win, kk_ap, acc_p[:, fi],
                                mybir.AluOpType.subtract, mybir.AluOpType.min,
                            )
                    first_pool = False
            if have_pool_acc:
                # absorber: collapse Pool's four acc_p slice deps to one
                _touch(acc_p[0:1, NF - 1, 0, 0, 0:1], {"v"})
                nc.vector.tensor_tensor(
                    acc_d[:], acc_p[:], acc_d[:], mybir.AluOpType.min
                )

        # Pool touch absorbs the DVE finalize dep (1 wait); the SWDGE out-DMA
        # dispatched next on the same sequencer then needs no data waits.
        _touch(acc_d[0:1, 0, 0, 0, 0:1], {"g"})
        nc.gpsimd.dma_start(yout[:], acc_d[:])

    return nc


def _get_program(repeat=1):
    key = repeat
    if key not in _prog_cache:
        _prog_cache[key] = _build_program(repeat)
    return _prog_cache[key]


def _krev(kernel):
    """[g, dy, dx, k, c, f] rotated/reversed SE, pure re-indexing of `kernel`."""
    k_ero = np.stack(
        [
            np.rot90(kernel[:, :, 2], k=3, axes=(0, 1)),
            kernel[:, :, 1],
            np.rot90(kernel[:, :, 0], k=1, axes=(0, 1)),
        ],
        axis=2,
    )
    krot = np.stack([np.rot90(k_ero, k=j, axes=(0, 1)) for j in range(4)], axis=0)
    return krot[:, ::-1, ::-1]


def _core_units(core):
    g = core // 2
    fh = core % 2
    return g, list(range(B)), list(range(fh * NF, fh * NF + NF))


def _make_in_map(x, kr, core):
    g, bs, fs = _core_units(core)
    # padded planes ygp[k][b, c, h', w'] (h', w' in [0, 100))
    xin = np.empty((15, 128, HB, NP, WP), np.float16)
    for k in range(3):
        src = x[:, (g + k - 1) % 4]  # [B, H, W, C]
        ygp = np.full((NP, C, H + 2 * PAD, WP), BIG, np.float32)
        for bi, b in enumerate(bs):
            ygp[bi, :, PAD : PAD + H, PAD : PAD + W] = src[b].transpose(2, 0, 1)
        for dy in range(KH):
            # tile[p=(c,hs), hb, b, wp] = ygp[b, c, hb*32+hs+dy, wp]
            v = ygp[:, :, dy : dy + H, :]  # [b, c, 96, 100]
            v = v.reshape(NP, C, HB, HS, WP)  # [b, c, hb, hs, wp]
            v = v.transpose(1, 3, 2, 0, 4)  # [c, hs, hb, b, wp]
            xin[k * KH + dy] = v.reshape(128, HB, NP, WP).astype(np.float16)
    # kk columns: +kk then -kk, col = ti*NF + fi, value kr[g, dy, dx, k, c(p), f]
    sel = kr[g][:, :, :, :, fs]  # [dy, dx, k, c, NF]
    tap_cf = np.ascontiguousarray(sel.transpose(2, 0, 1, 3, 4)).reshape(NTAP, C, NF)
    kk = np.empty((128, 2 * NTAP * NF), np.float32)
    for c in range(C):
        block = tap_cf[:, c, :].reshape(NTAP * NF)  # [ti*NF+fi]
        kk[c * HS : (c + 1) * HS, :NTAP * NF] = block[None, :]
        kk[c * HS : (c + 1) * HS, NTAP * NF :] = -block[None, :]
    return {"xin": xin, "kk": kk}


def _assemble(results):
    out = np.zeros((B, G, H, W, F), np.float32)
    for core in range(N_CORES):
        g, bs, fs = _core_units(core)
        y = np.asarray(results[core]["yout"]).astype(np.float32)
        # y[p=(c,hs), fi, hb, b, w] -> sum over c -> out[b, g, 32*hb+hs, w, f]
        y = y.reshape(C, HS, NF, HB, NP, W).sum(axis=0)  # [hs, fi, hb, b, w]
        y = y.transpose(3, 2, 0, 4, 1)  # [b, hb, hs, w, fi]
        y = y.reshape(NP, H, W, NF)
        for bi, b in enumerate(bs):
            out[b, g, :, :, fs[0] : fs[0] + NF] = y[bi]
    return out


def kernel(x, kernel):
    x = np.ascontiguousarray(np.asarray(x, dtype=np.float32))
    se = np.ascontiguousarray(np.asarray(kernel, dtype=np.float32))
    kr = _krev(se)  # [g, dy, dx, k, c, f]
    in_maps = [_make_in_map(x, kr, core) for core in range(N_CORES)]
    nc = _get_program(CFG_REPEAT)
    res = run_bass_kernel_spmd(nc, in_maps, list(range(N_CORES)), trace=False)
    global LAST_RESULTS
    LAST_RESULTS = res
    return _assemble(res.results)


# revision 16
# speedup vs baseline: 289.0401x; 96.0115x over previous
"""Trainium2 Bass kernel for ErosionP4 (P4 group-equivariant grayscale erosion).

Reference computation (shapes hardcoded):
  x: [B=4, G=4, H=96, W=96, C=4] fp32, kernel: [5, 5, 3, C=4, F=8] fp32
  out[b,g,h,w,f] = sum_c min_{k,dy,dx} ( ygp[b,g,k,h+dy,w+dx,c] - krev[g,dy,dx,k,c,f] )
  where ygp[b,g,k] = x[b, (g+k-1) mod 4] spatially padded with +inf and
  krev = the 4 planar rotations of the depth-rotated SE, spatially reversed.

Sharding: core -> (g = core//2, f-half = core%2).  Each core computes all 4
batches for one group-rotation g and 4 of the 8 filters.

Layout "c-block": partition p = 32*c + hs (c = channel, hs = h mod 32), free
dims (hb, b, w) with h = 32*hb + hs.  Each partition sees exactly one channel,
so the per-(tap, f) SE value is a per-partition scalar for FULL-width
tensor_scalar instructions (no chunk splitting).  The channel sum happens on
the host (c pieces live on different partitions).

The 15 (k, dy)-shifted input planes are pre-built on the host and DMA'd as 15
contiguous tiles via HWDGE on the otherwise-idle SP engine (SWDGE descriptor
generation would burn the Pool engine, which now does compute).

The 75 taps x 4 filters of acc = min(acc, window - kk) are split across three
engines (greedy balance by simulated per-engine cost):
  - DVE-own taps (even dx only, for 4B-aligned 4x packed reads):
      4x tensor_scalar subtract (4x mode) into a private tmp + one full-width
      (4 filters at once) tensor_tensor min (2x mode) into acc_dve.
  - ACT-assist taps: 4x activation(Identity, bias=-kk) subs into a ring slot,
      DVE does the full-width min into acc_dve (one cross-engine wait).
  - Pool taps: 4x fused scalar_tensor_tensor (subtract, min) into acc_pool
      (the Q7 software implementation fuses both ALU stages at no extra cost).
Finally acc = min(acc_dve, acc_pool) on DVE and one HWDGE store of the
[128, 4, 1152] fp16 result.
"""

import os
from contextlib import ExitStack

import numpy as np

import concourse.bass as bass
import concourse.mybir as mybir
import concourse.tile as tile
from concourse.bass_utils import run_bass_kernel_spmd

B, G, H, W, C = 4, 4, 96, 96, 4
KH, KW, F = 5, 5, 8
PAD = 2
WP = W + PAD * 2  # 100
NTAP = 3 * KH * KW  # 75
N_CORES = 8
NP = 4  # batches per core
NF = F // 2  # filters per core
HS = 32  # h rows per (c, hb) block
HB = 3  # h blocks
BIG = 30000.0  # +inf stand-in that survives fp16

CFG_REPEAT = int(os.environ.get("KCFG_REPEAT", "1"))
# simulated per-unit costs (ns) used by the static scheduler.  The Pool
# engine is DMA-only on this walrus backend (all its elementwise tensor ops
# fail the codegen engine check), so compute is split DVE vs ACT only.
COST_DVE_SUB = 360.0
COST_DVE_MIN_WIDE = 2460.0
COST_ACT_SUB = 1185.0  # activation + amortized absorber copies

FP16 = mybir.dt.float16

_prog_cache = {}
LAST_RESULTS = None


def _taps():
    return [(k, dy, dx) for k in range(3) for dy in range(KH) for dx in range(KW)]


def _schedule():
    """Greedy static assignment of the 75 taps to engines.

    Returns list of 'dve' | 'act' | 'pool' per tap index.  DVE-own taps are
    restricted to even dx (aligned 4x packed reads); the first tap must be
    'dve' so acc_dve exists before any assist-min, and the first 'pool' tap
    initializes acc_pool.
    """
    taps = _taps()
    t_dve = t_act = 0.0
    out = []
    for ti, (k, dy, dx) in enumerate(taps):
        # projected end-times if this tap went to each engine
        cand = []
        if dx % 2 == 0:
            cand.append(("dve", t_dve + NF * COST_DVE_SUB + COST_DVE_MIN_WIDE))
        cand.append(("act", max(t_act + NF * COST_ACT_SUB,
                                t_dve + COST_DVE_MIN_WIDE)))
        if ti == 0:
            cand = [c for c in cand if c[0] == "dve"]
        eng, _ = min(cand, key=lambda c: c[1])
        if eng == "dve":
            t_dve += NF * COST_DVE_SUB + COST_DVE_MIN_WIDE
        else:
            t_act = max(t_act, 0.0) + NF * COST_ACT_SUB
            t_dve += COST_DVE_MIN_WIDE
        out.append(eng)
    return out


class _SplitDrainTC(tile.TileContext):
    """TileContext whose kernel-tail drain is split into one drain per sem
    lane: the stock single Drain carries a wait for every lane used, which
    overflows the CTRL struct's sync-wait encoding on this compiler."""

    def _drain_and_barrier(self, tick_clock, wait_clock):
        from concourse.tile_sem_assignment import N_PROCS
        from concourse.vector_clock import ScopedClock, VectorClock

        gc = tick_clock.global_clock
        ticks = [gc[p] for p in range(N_PROCS)]
        for p in range(N_PROCS):
            if ticks[p] <= 0:
                continue
            sub = [ticks[q] if q == p else 0 for q in range(N_PROCS)]
            d = self.nc.sync.drain()
            wait_clock.add_sem_waits(d.ins, ScopedClock({None: VectorClock(sub)}))

        self.nc.all_engine_barrier()
        assert self.sems is not None
        popped = self.nc._tile_sem_poison_stack.pop()
        assert popped is self._sem_poison
        self.nc.clear_and_free_semaphores(list(self.sems.allocated().values()))
        self.nc.all_engine_barrier()


def _strip_stale_same_engine_waits(nc, lag=8):
    """Drop same-engine sem waits whose producer finished >= `lag` own-engine
    instructions earlier.

    This tile version emits a sem wait for EVERY hazard, including same-engine
    WAW/WAR whose producers are long retired; compute ISA structs can encode
    only ONE sync wait, so a ring-buffer rewrite (same-engine WAW + cross-
    engine WAR) overflows codegen.  Engines issue in order and their writes
    land within a couple of instructions, so a same-engine wait on a producer
    `lag` instructions back is vacuous.  Recent same-engine waits (pipelined
    RAW guards) are kept.
    """
    strip_types = {
        "InstActivation", "InstTensorScalarPtr", "InstTensorTensor",
        "InstTensorScalar", "InstMemset", "InstCopy", "InstTensorCopy",
        "InstTensorReduce",
    }
    counts = {}
    fn = nc.m.functions[0]
    for blk in fn.blocks:
        for ins in blk.instructions:
            si = ins.sync_info
            if si is None:
                continue
            eng = getattr(ins, "engine", None)
            ename = getattr(eng, "name", None) or (str(eng).split(".")[-1] if eng else "")
            if si.on_wait and type(ins).__name__ in strip_types and ename in (
                "Activation", "DVE", "Pool", "PE"
            ):
                keep = []
                for w in si.on_wait:
                    nm = w.ant_name or ""
                    if (
                        nm.startswith(ename + "_")
                        and w.wait_mode == "sem-ge-imm"
                        and w.wait_value is not None
                        and counts.get(nm, 0) - w.wait_value >= lag
                    ):
                        continue
                    keep.append(w)
                if len(keep) != len(si.on_wait):
                    si.on_wait = keep
            for u in si.on_update or []:
                if u.ant_name:
                    counts[u.ant_name] = counts.get(u.ant_name, 0) + (u.update_value or 1)
    return nc


def _build_program(repeat=1):
    import concourse.tile_sem_assignment as _tsa

    _orig_swdge = _tsa.NUM_SWDGE_GLOBAL_SEMS
    _tsa.NUM_SWDGE_GLOBAL_SEMS = 4
    try:
        return _strip_stale_same_engine_waits(_build_program_inner(repeat))
    finally:
        _tsa.NUM_SWDGE_GLOBAL_SEMS = _orig_swdge


def _build_program_inner(repeat=1):
    nc = bass.Bass()
    sched = _schedule()
    taps = _taps()

    # xin2[t15][p][hb][b][wp]: host-pre-shifted planes, one contiguous tile per
    # (k, dy).  kk: +kk columns then -kk columns (ACT bias), col = ti*NF + fi.
    xin = nc.declare_dram_parameter("xin", [15, 128, HB, NP, WP], FP16, isOutput=False)
    kkin = nc.declare_dram_parameter("kk", [128, 2 * NTAP * NF], mybir.dt.float32, isOutput=False)
    yout = nc.declare_dram_parameter("yout", [128, NF, HB, NP, W], FP16, isOutput=True)

    with _SplitDrainTC(nc) as tc, ExitStack() as ctx:
        pool = ctx.enter_context(tc.tile_pool(name="main", bufs=1))

        # Compute-instruction ISA slots can encode only ONE sync wait, so
        # "touch" every DMA'd region with a trivial op on each consuming
        # engine right after its DMA; later compute instructions inherit the
        # dependency through engine program order and carry no waits.
        touch_v = pool.tile([1, 256], mybir.dt.float32, name="touch_v", tag="touch_v")
        touch_s = pool.tile([1, 256], mybir.dt.float32, name="touch_s", tag="touch_s")
        touch_g = pool.tile([1, 256], mybir.dt.float32, name="touch_g", tag="touch_g")
        tctr = [0, 0, 0]

        def _touch(src, engines):
            # columns cycle mod 256: the WAW producer of a reused column is
            # hundreds of instructions back, so its same-engine wait is
            # stripped by _strip_stale_same_engine_waits.
            if "v" in engines:
                tctr[0] += 1
                i = tctr[0] % 256
                nc.vector.tensor_scalar_add(touch_v[0:1, i : i + 1], src, 0.0)
            if "s" in engines:
                tctr[1] += 1
                i = tctr[1] % 256
                nc.scalar.copy(touch_s[0:1, i : i + 1], src)
            if "g" in engines:
                tctr[2] += 1
                i = tctr[2] % 256
                nc.gpsimd.tensor_scalar_add(touch_g[0:1, i : i + 1], src, 0.0)

        # which engines read each (k, dy) tile
        tile_readers = {}
        for ti, (k, dy, dx) in enumerate(taps):
            e = {"dve": "v", "act": "s", "pool": "g"}[sched[ti]]
            tile_readers.setdefault((k, dy), set()).add(e)

        in_t = {}
        for k in range(3):
            for dy in range(KH):
                t = pool.tile([128, HB, NP, WP], FP16, name=f"in_{k}_{dy}", tag=f"in_{k}_{dy}")
                nc.sync.dma_start(t[:], xin[(k * KH + dy)])
                _touch(t[0:1, 0, 0, 0:1], tile_readers[(k, dy)])
                in_t[k, dy] = t

        kkt = pool.tile([128, 2 * NTAP * NF], mybir.dt.float32, name="kkt", tag="kkt")
        nc.sync.dma_start(kkt[:], kkin[:])
        _touch(kkt[0:1, 0:1], {"v", "s", "g"})

        acc_d = pool.tile([128, NF, HB, NP, W], FP16, name="acc_d", tag="acc_d")
        acc_p = pool.tile([128, NF, HB, NP, W], FP16, name="acc_p", tag="acc_p")
        tmp_d = pool.tile([128, NF, HB, NP, W], FP16, name="tmp_d", tag="tmp_d")
        NRING = 6
        ring = [
            pool.tile([128, NF, HB, NP, W], FP16, name=f"ring_{i}", tag=f"ring_{i}")
            for i in range(NRING)
        ]

        have_pool_acc = any(e == "pool" for e in sched)

        ring_i = 0
        assist_marks = []
        for _rep in range(repeat):
            first_pool = True
            # touch_v column written by the DVE mark after each assist-min;
            # the ACT absorber for a recycled ring slot reads the mark of the
            # min that read that slot (cross-engine RAW, one wait on a copy),
            # avoiding any ACT read of acc_d (which would put a WAR wait on
            # the next acc-writing TT, whose struct has no room for it).
            for ti, (k, dy, dx) in enumerate(taps):
                eng = sched[ti]
                src = in_t[k, dy]
                win = src[:, :, :, dx : dx + W]
                if eng == "dve":
                    dst = acc_d if ti == 0 else tmp_d
                    for fi in range(NF):
                        kk_ap = kkt[:, ti * NF + fi : ti * NF + fi + 1]
                        nc.vector.tensor_scalar(
                            dst[:, fi], win, kk_ap, None, mybir.AluOpType.subtract
                        )
                    if ti != 0:
                        nc.vector.tensor_tensor(
                            acc_d[:], tmp_d[:], acc_d[:], mybir.AluOpType.min
                        )
                elif eng == "act":
                    slot = ring[ring_i % NRING]
                    if ring_i >= NRING:
                        # ACT absorber: observe the DVE mark of the min that
                        # read this slot, so the ring rewrites carry no WAR
                        # waits (the AC struct cannot encode them).
                        mark_col = assist_marks[ring_i - NRING]
                        _touch(touch_v[0:1, mark_col : mark_col + 1], {"s"})
                    ring_i += 1
                    for fi in range(NF):
                        negkk_ap = kkt[:, NTAP * NF + ti * NF + fi : NTAP * NF + ti * NF + fi + 1]
                        nc.scalar.activation(
                            slot[:, fi], win,
                            mybir.ActivationFunctionType.Identity, bias=negkk_ap,
                        )
                    # DVE absorber on the last sub: the wide min then inherits
                    # all four slice deps through DVE program order (TT structs
                    # encode only one sync wait).
                    _touch(slot[0:1, NF - 1, 0, 0, 0:1], {"v"})
                    nc.vector.tensor_tensor(
                        acc_d[:], slot[:], acc_d[:], mybir.AluOpType.min
                    )
                    # DVE mark: same-engine read of acc_d right after the min
                    tctr[0] += 1
                    i = tctr[0] % 256
                    nc.vector.tensor_scalar_add(
                        touch_v[0:1, i : i + 1], acc_d[0:1, 0, 0, 0, 0:1], 0.0
                    )
                    assist_marks.append(i)
                else:  # pool
                    for fi in range(NF):
                        kk_ap = kkt[:, ti * NF + fi : ti * NF + fi + 1]
                        if first_pool:
                            nc.gpsimd.tensor_scalar(
                                acc_p[:, fi], win, kk_ap, None, mybir.AluOpType.subtract
                            )
                        else:
                            nc.gpsimd.scalar_tensor_tensor(
                                acc_p[:, fi], win, kk_ap, acc_p[:, fi],
                                mybir.AluOpType.subtract, mybir.AluOpType.min,
                            )
                    first_pool = False
            if have_pool_acc:
                # absorber: collapse Pool's four acc_p slice deps to one
                _touch(acc_p[0:1, NF - 1, 0, 0, 0:1], {"v"})
                nc.vector.tensor_tensor(
                    acc_d[:], acc_p[:], acc_d[:], mybir.AluOpType.min
                )

        # Pool touch absorbs the DVE finalize dep (1 wait); the SWDGE out-DMA
        # dispatched next on the same sequencer then needs no data waits.
        _touch(acc_d[0:1, 0, 0, 0, 0:1], {"g"})
        nc.gpsimd.dma_start(yout[:], acc_d[:])

    return nc


def _get_program(repeat=1):
    key = repeat
    if key not in _prog_cache:
        _prog_cache[key] = _build_program(repeat)
    return _prog_cache[key]


def _krev(kernel):
    """[g, dy, dx, k, c, f] rotated/reversed SE, pure re-indexing of `kernel`."""
    k_ero = np.stack(
        [
            np.rot90(kernel[:, :, 2], k=3, axes=(0, 1)),
            kernel[:, :, 1],
            np.rot90(kernel[:, :, 0], k=1, axes=(0, 1)),
        ],
        axis=2,
    )
    krot = np.stack([np.rot90(k_ero, k=j, axes=(0, 1)) for j in range(4)], axis=0)
    return krot[:, ::-1, ::-1]


def _core_units(core):
    g = core // 2
    fh = core % 2
    return g, list(range(B)), list(range(fh * NF, fh * NF + NF))


def _make_in_map(x, kr, core):
    g, bs, fs = _core_units(core)
    # padded planes ygp[k][b, c, h', w'] (h', w' in [0, 100))
    xin = np.empty((15, 128, HB, NP, WP), np.float16)
    for k in range(3):
        src = x[:, (g + k - 1) % 4]  # [B, H, W, C]
        ygp = np.full((NP, C, H + 2 * PAD, WP), BIG, np.float32)
        for bi, b in enumerate(bs):
            ygp[bi, :, PAD : PAD + H, PAD : PAD + W] = src[b].transpose(2, 0, 1)
        for dy in range(KH):
            # tile[p=(c,hs), hb, b, wp] = ygp[b, c, hb*32+hs+dy, wp]
            v = ygp[:, :, dy : dy + H, :]  # [b, c, 96, 100]
            v = v.reshape(NP, C, HB, HS, WP)  # [b, c, hb, hs, wp]
            v = v.transpose(1, 3, 2, 0, 4)  # [c, hs, hb, b, wp]
            xin[k * KH + dy] = v.reshape(128, HB, NP, WP).astype(np.float16)
    # kk columns: +kk then -kk, col = ti*NF + fi, value kr[g, dy, dx, k, c(p), f]
    sel = kr[g][:, :, :, :, fs]  # [dy, dx, k, c, NF]
    tap_cf = np.ascontiguousarray(sel.transpose(2, 0, 1, 3, 4)).reshape(NTAP, C, NF)
    kk = np.empty((128, 2 * NTAP * NF), np.float32)
    for c in range(C):
        block = tap_cf[:, c, :].reshape(NTAP * NF)  # [ti*NF+fi]
        kk[c * HS : (c + 1) * HS, :NTAP * NF] = block[None, :]
        kk[c * HS : (c + 1) * HS, NTAP * NF :] = -block[None, :]
    return {"xin": xin, "kk": kk}


def _assemble(results):
    out = np.zeros((B, G, H, W, F), np.float32)
    for core in range(N_CORES):
        g, bs, fs = _core_units(core)
        y = np.asarray(results[core]["yout"]).astype(np.float32)
        # y[p=(c,hs), fi, hb, b, w] -> sum over c -> out[b, g, 32*hb+hs, w, f]
        y = y.reshape(C, HS, NF, HB, NP, W).sum(axis=0)  # [hs, fi, hb, b, w]
        y = y.transpose(3, 2, 0, 4, 1)  # [b, hb, hs, w, fi]
        y = y.reshape(NP, H, W, NF)
        for bi, b in enumerate(bs):
            out[b, g, :, :, fs[0] : fs[0] + NF] = y[bi]
    return out


def kernel(x, kernel):
    x = np.ascontiguousarray(np.asarray(x, dtype=np.float32))
    se = np.ascontiguousarray(np.asarray(kernel, dtype=np.float32))
    kr = _krev(se)  # [g, dy, dx, k, c, f]
    in_maps = [_make_in_map(x, kr, core) for core in range(N_CORES)]
    nc = _get_program(CFG_REPEAT)
    res = run_bass_kernel_spmd(nc, in_maps, list(range(N_CORES)), trace=False)
    global LAST_RESULTS
    LAST_RESULTS = res
    return _assemble(res.results)


# revision 18
# speedup vs baseline: 380.4400x; 1.3162x over previous
"""Trainium2 Bass kernel for ErosionP4 (P4 group-equivariant grayscale erosion).

Reference computation (shapes hardcoded):
  x: [B=4, G=4, H=96, W=96, C=4] fp32, kernel: [5, 5, 3, C=4, F=8] fp32
  out[b,g,h,w,f] = sum_c min_{k,dy,dx} ( ygp[b,g,k,h+dy,w+dx,c] - krev[g,dy,dx,k,c,f] )
  where ygp[b,g,k] = x[b, (g+k-1) mod 4] spatially padded with +inf and
  krev = the 4 planar rotations of the depth-rotated SE, spatially reversed.

Sharding: core -> (g = core//2, f-half = core%2).  Each core computes all 4
batches for one group-rotation g and 4 of the 8 filters.

Layout "c-block": partition p = 32*c + hs (c = channel, hs = h mod 32), free
dims (hb, b, w) with h = 32*hb + hs.  Each partition sees exactly one channel,
so the per-(tap, f) SE value is a per-partition scalar for FULL-width
tensor_scalar instructions (no chunk splitting).  The channel sum happens on
the host (c pieces live on different partitions).

The 15 (k, dy)-shifted input planes are pre-built on the host and DMA'd as 15
contiguous tiles via HWDGE on the otherwise-idle SP engine (SWDGE descriptor
generation would burn the Pool engine, which now does compute).

The 75 taps x 4 filters of acc = min(acc, window - kk) are split across three
engines (greedy balance by simulated per-engine cost):
  - DVE-own taps (even dx only, for 4B-aligned 4x packed reads):
      4x tensor_scalar subtract (4x mode) into a private tmp + one full-width
      (4 filters at once) tensor_tensor min (2x mode) into acc_dve.
  - ACT-assist taps: 4x activation(Identity, bias=-kk) subs into a ring slot,
      DVE does the full-width min into acc_dve (one cross-engine wait).
  - Pool taps: 4x fused scalar_tensor_tensor (subtract, min) into acc_pool
      (the Q7 software implementation fuses both ALU stages at no extra cost).
Finally acc = min(acc_dve, acc_pool) on DVE and one HWDGE store of the
[128, 4, 1152] fp16 result.
"""

import os
from contextlib import ExitStack

import numpy as np

import concourse.bass as bass
import concourse.mybir as mybir
import concourse.tile as tile
from concourse.bass_utils import run_bass_kernel_spmd

B, G, H, W, C = 4, 4, 96, 96, 4
KH, KW, F = 5, 5, 8
PAD = 2
WP = W + PAD * 2  # 100
NTAP = 3 * KH * KW  # 75
N_CORES = 8
NP = 4  # batches per core
NF = F // 2  # filters per core
HS = 32  # h rows per (c, hb) block
HB = 3  # h blocks
BIG = 30000.0  # +inf stand-in that survives fp16

CFG_REPEAT = int(os.environ.get("KCFG_REPEAT", "1"))
# simulated per-unit costs (ns) used by the static scheduler.  The Pool
# engine is DMA-only on this walrus backend (all its elementwise tensor ops
# fail the codegen engine check), so compute is split DVE vs ACT only.
COST_DVE_SUB = 360.0
COST_DVE_MIN_WIDE = 2460.0
# activation + amortized absorber copies; KCFG_ACT_COST recalibrates the
# DVE/ACT split against real hardware ratios (higher -> fewer ACT taps)
COST_ACT_SUB = float(os.environ.get("KCFG_ACT_COST", "1400"))

FP16 = mybir.dt.float16

_prog_cache = {}
LAST_RESULTS = None


def _taps():
    return [(k, dy, dx) for k in range(3) for dy in range(KH) for dx in range(KW)]


def _schedule():
    """Greedy static assignment of the 75 taps to engines.

    Returns list of 'dve' | 'act' | 'pool' per tap index.  DVE-own taps are
    restricted to even dx (aligned 4x packed reads); the first tap must be
    'dve' so acc_dve exists before any assist-min, and the first 'pool' tap
    initializes acc_pool.
    """
    taps = _taps()
    t_dve = t_act = 0.0
    out = []
    for ti, (k, dy, dx) in enumerate(taps):
        # projected end-times if this tap went to each engine
        cand = []
        if dx % 2 == 0:
            cand.append(("dve", t_dve + NF * COST_DVE_SUB + COST_DVE_MIN_WIDE))
        cand.append(("act", max(t_act + NF * COST_ACT_SUB,
                                t_dve + COST_DVE_MIN_WIDE)))
        if ti == 0:
            cand = [c for c in cand if c[0] == "dve"]
        eng, _ = min(cand, key=lambda c: c[1])
        if eng == "dve":
            t_dve += NF * COST_DVE_SUB + COST_DVE_MIN_WIDE
        else:
            t_act = max(t_act, 0.0) + NF * COST_ACT_SUB
            t_dve += COST_DVE_MIN_WIDE
        out.append(eng)
    return out


class _SplitDrainTC(tile.TileContext):
    """TileContext whose kernel-tail drain is split into one drain per sem
    lane: the stock single Drain carries a wait for every lane used, which
    overflows the CTRL struct's sync-wait encoding on this compiler."""

    def _drain_and_barrier(self, tick_clock, wait_clock):
        from concourse.tile_sem_assignment import N_PROCS
        from concourse.vector_clock import ScopedClock, VectorClock

        gc = tick_clock.global_clock
        ticks = [gc[p] for p in range(N_PROCS)]
        for p in range(N_PROCS):
            if ticks[p] <= 0:
                continue
            sub = [ticks[q] if q == p else 0 for q in range(N_PROCS)]
            d = self.nc.sync.drain()
            wait_clock.add_sem_waits(d.ins, ScopedClock({None: VectorClock(sub)}))

        self.nc.all_engine_barrier()
        assert self.sems is not None
        popped = self.nc._tile_sem_poison_stack.pop()
        assert popped is self._sem_poison
        self.nc.clear_and_free_semaphores(list(self.sems.allocated().values()))
        self.nc.all_engine_barrier()


def _strip_stale_same_engine_waits(nc, lag=8):
    """Drop same-engine sem waits whose producer finished >= `lag` own-engine
    instructions earlier.

    This tile version emits a sem wait for EVERY hazard, including same-engine
    WAW/WAR whose producers are long retired; compute ISA structs can encode
    only ONE sync wait, so a ring-buffer rewrite (same-engine WAW + cross-
    engine WAR) overflows codegen.  Engines issue in order and their writes
    land within a couple of instructions, so a same-engine wait on a producer
    `lag` instructions back is vacuous.  Recent same-engine waits (pipelined
    RAW guards) are kept.
    """
    strip_types = {
        "InstActivation", "InstTensorScalarPtr", "InstTensorTensor",
        "InstTensorScalar", "InstMemset", "InstCopy", "InstTensorCopy",
        "InstTensorReduce",
    }
    counts = {}
    fn = nc.m.functions[0]
    for blk in fn.blocks:
        for ins in blk.instructions:
            si = ins.sync_info
            if si is None:
                continue
            eng = getattr(ins, "engine", None)
            ename = getattr(eng, "name", None) or (str(eng).split(".")[-1] if eng else "")
            if si.on_wait and type(ins).__name__ in strip_types and ename in (
                "Activation", "DVE", "Pool", "PE"
            ):
                keep = []
                for w in si.on_wait:
                    nm = w.ant_name or ""
                    if (
                        nm.startswith(ename + "_")
                        and w.wait_mode == "sem-ge-imm"
                        and w.wait_value is not None
                        and counts.get(nm, 0) - w.wait_value >= lag
                    ):
                        continue
                    keep.append(w)
                if len(keep) != len(si.on_wait):
                    si.on_wait = keep
            for u in si.on_update or []:
                if u.ant_name:
                    counts[u.ant_name] = counts.get(u.ant_name, 0) + (u.update_value or 1)
    return nc


def _build_program(repeat=1):
    import concourse.tile_sem_assignment as _tsa

    _orig_swdge = _tsa.NUM_SWDGE_GLOBAL_SEMS
    _tsa.NUM_SWDGE_GLOBAL_SEMS = 4
    try:
        return _strip_stale_same_engine_waits(_build_program_inner(repeat))
    finally:
        _tsa.NUM_SWDGE_GLOBAL_SEMS = _orig_swdge


def _build_program_inner(repeat=1):
    nc = bass.Bass()
    sched = _schedule()
    taps = _taps()

    # xin2[t15][p][hb][b][wp]: host-pre-shifted planes, one contiguous tile per
    # (k, dy).  kk: +kk columns then -kk columns (ACT bias), col = ti*NF + fi.
    xin = nc.declare_dram_parameter("xin", [15, 128, HB, NP, WP], FP16, isOutput=False)
    kkin = nc.declare_dram_parameter("kk", [128, 2 * NTAP * NF], mybir.dt.float32, isOutput=False)
    yout = nc.declare_dram_parameter("yout", [128, NF, HB, NP, W], FP16, isOutput=True)

    with _SplitDrainTC(nc) as tc, ExitStack() as ctx:
        pool = ctx.enter_context(tc.tile_pool(name="main", bufs=1))

        # Compute-instruction ISA slots can encode only ONE sync wait, so
        # "touch" every DMA'd region with a trivial op on each consuming
        # engine right after its DMA; later compute instructions inherit the
        # dependency through engine program order and carry no waits.
        touch_v = pool.tile([1, 256], mybir.dt.float32, name="touch_v", tag="touch_v")
        touch_s = pool.tile([1, 256], mybir.dt.float32, name="touch_s", tag="touch_s")
        touch_g = pool.tile([1, 256], mybir.dt.float32, name="touch_g", tag="touch_g")
        tctr = [0, 0, 0]

        def _touch(src, engines):
            # columns cycle mod 256: the WAW producer of a reused column is
            # hundreds of instructions back, so its same-engine wait is
            # stripped by _strip_stale_same_engine_waits.
            if "v" in engines:
                tctr[0] += 1
                i = tctr[0] % 256
                nc.vector.tensor_scalar_add(touch_v[0:1, i : i + 1], src, 0.0)
            if "s" in engines:
                tctr[1] += 1
                i = tctr[1] % 256
                nc.scalar.copy(touch_s[0:1, i : i + 1], src)
            if "g" in engines:
                tctr[2] += 1
                i = tctr[2] % 256
                nc.gpsimd.tensor_scalar_add(touch_g[0:1, i : i + 1], src, 0.0)

        # which engines read each (k, dy) tile
        tile_readers = {}
        for ti, (k, dy, dx) in enumerate(taps):
            e = {"dve": "v", "act": "s", "pool": "g"}[sched[ti]]
            tile_readers.setdefault((k, dy), set()).add(e)

        # kkt first: every tap's first instruction reads it, so it must land
        # before any compute can start.
        kkt = pool.tile([128, 2 * NTAP * NF], mybir.dt.float32, name="kkt", tag="kkt")
        nc.sync.dma_start(kkt[:], kkin[:])
        _touch(kkt[0:1, 0:1], {"v", "s", "g"})

        in_t = {}
        for k in range(3):
            for dy in range(KH):
                t = pool.tile([128, HB, NP, WP], FP16, name=f"in_{k}_{dy}", tag=f"in_{k}_{dy}")
                nc.sync.dma_start(t[:], xin[(k * KH + dy)])
                _touch(t[0:1, 0, 0, 0:1], tile_readers[(k, dy)])
                in_t[k, dy] = t

        acc_d = pool.tile([128, NF, HB, NP, W], FP16, name="acc_d", tag="acc_d")
        acc_p = pool.tile([128, NF, HB, NP, W], FP16, name="acc_p", tag="acc_p")
        tmp_d = pool.tile([128, NF, HB, NP, W], FP16, name="tmp_d", tag="tmp_d")
        NRING = 6
        ring = [
            pool.tile([128, NF, HB, NP, W], FP16, name=f"ring_{i}", tag=f"ring_{i}")
            for i in range(NRING)
        ]

        have_pool_acc = any(e == "pool" for e in sched)

        ring_i = 0
        assist_marks = []
        for _rep in range(repeat):
            first_pool = True
            # touch_v column written by the DVE mark after each assist-min;
            # the ACT absorber for a recycled ring slot reads the mark of the
            # min that read that slot (cross-engine RAW, one wait on a copy),
            # avoiding any ACT read of acc_d (which would put a WAR wait on
            # the next acc-writing TT, whose struct has no room for it).
            for ti, (k, dy, dx) in enumerate(taps):
                eng = sched[ti]
                src = in_t[k, dy]
                win = src[:, :, :, dx : dx + W]
                if eng == "dve":
                    dst = acc_d if ti == 0 else tmp_d
                    for fi in range(NF):
                        kk_ap = kkt[:, ti * NF + fi : ti * NF + fi + 1]
                        nc.vector.tensor_scalar(
                            dst[:, fi], win, kk_ap, None, mybir.AluOpType.subtract
                        )
                    if ti != 0:
                        nc.vector.tensor_tensor(
                            acc_d[:], tmp_d[:], acc_d[:], mybir.AluOpType.min
                        )
                elif eng == "act":
                    slot = ring[ring_i % NRING]
                    if ring_i >= NRING:
                        # ACT absorber: observe the DVE mark of the min that
                        # read this slot, so the ring rewrites carry no WAR
                        # waits (the AC struct cannot encode them).
                        mark_col = assist_marks[ring_i - NRING]
                        _touch(touch_v[0:1, mark_col : mark_col + 1], {"s"})
                    ring_i += 1
                    for fi in range(NF):
                        negkk_ap = kkt[:, NTAP * NF + ti * NF + fi : NTAP * NF + ti * NF + fi + 1]
                        nc.scalar.activation(
                            slot[:, fi], win,
                            mybir.ActivationFunctionType.Identity, bias=negkk_ap,
                        )
                    # DVE absorber on the last sub: the wide min then inherits
                    # all four slice deps through DVE program order (TT structs
                    # encode only one sync wait).
                    _touch(slot[0:1, NF - 1, 0, 0, 0:1], {"v"})
                    nc.vector.tensor_tensor(
                        acc_d[:], slot[:], acc_d[:], mybir.AluOpType.min
                    )
                    # DVE mark: same-engine read of acc_d right after the min
                    tctr[0] += 1
                    i = tctr[0] % 256
                    nc.vector.tensor_scalar_add(
                        touch_v[0:1, i : i + 1], acc_d[0:1, 0, 0, 0, 0:1], 0.0
                    )
                    assist_marks.append(i)
                else:  # pool
                    for fi in range(NF):
                        kk_ap = kkt[:, ti * NF + fi : ti * NF + fi + 1]
                        if first_pool:
                            nc.gpsimd.tensor_scalar(
                                acc_p[:, fi], win, kk_ap, None, mybir.AluOpType.subtract
                            )
                        else:
                            nc.gpsimd.scalar_tensor_tensor(
                                acc_p[:, fi], win, kk_ap, acc_p[:, fi],
                                mybir.AluOpType.subtract, mybir.AluOpType.min,
                            )
                    first_pool = False
            if have_pool_acc:
                # absorber: collapse Pool's four acc_p slice deps to one
                _touch(acc_p[0:1, NF - 1, 0, 0, 0:1], {"v"})
                nc.vector.tensor_tensor(
                    acc_d[:], acc_p[:], acc_d[:], mybir.AluOpType.min
                )

        # Pool touch absorbs the DVE finalize dep (1 wait); the SWDGE out-DMA
        # dispatched next on the same sequencer then needs no data waits.
        _touch(acc_d[0:1, 0, 0, 0, 0:1], {"g"})
        nc.gpsimd.dma_start(yout[:], acc_d[:])

    return nc


def _get_program(repeat=1):
    key = repeat
    if key not in _prog_cache:
        _prog_cache[key] = _build_program(repeat)
    return _prog_cache[key]


def _krev(kernel):
    """[g, dy, dx, k, c, f] rotated/reversed SE, pure re-indexing of `kernel`."""
    k_ero = np.stack(
        [
            np.rot90(kernel[:, :, 2], k=3, axes=(0, 1)),
            kernel[:, :, 1],
            np.rot90(kernel[:, :, 0], k=1, axes=(0, 1)),
        ],
        axis=2,
    )
    krot = np.stack([np.rot90(k_ero, k=j, axes=(0, 1)) for j in range(4)], axis=0)
    return krot[:, ::-1, ::-1]


def _core_units(core):
    g = core // 2
    fh = core % 2
    return g, list(range(B)), list(range(fh * NF, fh * NF + NF))


def _make_in_map(x, kr, core):
    g, bs, fs = _core_units(core)
    # padded planes ygp[k][b, c, h', w'] (h', w' in [0, 100))
    xin = np.empty((15, 128, HB, NP, WP), np.float16)
    for k in range(3):
        src = x[:, (g + k - 1) % 4]  # [B, H, W, C]
        ygp = np.full((NP, C, H + 2 * PAD, WP), BIG, np.float32)
        for bi, b in enumerate(bs):
            ygp[bi, :, PAD : PAD + H, PAD : PAD + W] = src[b].transpose(2, 0, 1)
        for dy in range(KH):
            # tile[p=(c,hs), hb, b, wp] = ygp[b, c, hb*32+hs+dy, wp]
            v = ygp[:, :, dy : dy + H, :]  # [b, c, 96, 100]
            v = v.reshape(NP, C, HB, HS, WP)  # [b, c, hb, hs, wp]
            v = v.transpose(1, 3, 2, 0, 4)  # [c, hs, hb, b, wp]
            xin[k * KH + dy] = v.reshape(128, HB, NP, WP).astype(np.float16)
    # kk columns: +kk then -kk, col = ti*NF + fi, value kr[g, dy, dx, k, c(p), f]
    sel = kr[g][:, :, :, :, fs]  # [dy, dx, k, c, NF]
    tap_cf = np.ascontiguousarray(sel.transpose(2, 0, 1, 3, 4)).reshape(NTAP, C, NF)
    kk = np.empty((128, 2 * NTAP * NF), np.float32)
    for c in range(C):
        block = tap_cf[:, c, :].reshape(NTAP * NF)  # [ti*NF+fi]
        kk[c * HS : (c + 1) * HS, :NTAP * NF] = block[None, :]
        kk[c * HS : (c + 1) * HS, NTAP * NF :] = -block[None, :]
    return {"xin": xin, "kk": kk}


def _assemble(results):
    out = np.zeros((B, G, H, W, F), np.float32)
    for core in range(N_CORES):
        g, bs, fs = _core_units(core)
        y = np.asarray(results[core]["yout"]).astype(np.float32)
        # y[p=(c,hs), fi, hb, b, w] -> sum over c -> out[b, g, 32*hb+hs, w, f]
        y = y.reshape(C, HS, NF, HB, NP, W).sum(axis=0)  # [hs, fi, hb, b, w]
        y = y.transpose(3, 2, 0, 4, 1)  # [b, hb, hs, w, fi]
        y = y.reshape(NP, H, W, NF)
        for bi, b in enumerate(bs):
            out[b, g, :, :, fs[0] : fs[0] + NF] = y[bi]
    return out


def kernel(x, kernel):
    x = np.ascontiguousarray(np.asarray(x, dtype=np.float32))
    se = np.ascontiguousarray(np.asarray(kernel, dtype=np.float32))
    kr = _krev(se)  # [g, dy, dx, k, c, f]
    in_maps = [_make_in_map(x, kr, core) for core in range(N_CORES)]
    nc = _get_program(CFG_REPEAT)
    res = run_bass_kernel_spmd(nc, in_maps, list(range(N_CORES)), trace=False)
    global LAST_RESULTS
    LAST_RESULTS = res
    return _assemble(res.results)
